# revision 1
# baseline (speedup 1.0000x reference)
"""BallQueryAttention TRN2 kernel.

Math: reference computes softmax over a binary ball mask (d2 <= R^2), then
mask-softmax @ x.  exp of a 0/1 mask takes only values {1, e}, so

  out[i] = (S + (e-1) * sum_{j in ball(i)} x_j) / (N + (e-1) * cnt_i)

with S = colsum(x).  Sharding: rows (i) across 8 cores, x replicated.

Per core (row shard of 1024):
  - Gram tiles Gt[j_tile(128 part), i(1024 free)] via 2 fp16 hi/lo-split
    matmuls (fp32-class accuracy on the distance threshold).  The sq_i term
    rides 3 fp16-split augmentation rows on the moving operand; the sq_j
    term rides fp32 per-partition bias/threshold in the compare op.
  - mask compare split across Vector (is_ge -> {0,2}) and Scalar
    (Sign -> {-1,1}) engines, writing fp16 masks.
  - accumulating [x|1]^T @ mask matmul -> [65, 1024] in PSUM, plus
    ones-column sums (SALL over all tiles, SAO over sign tiles) to undo
    the {0,2}/{-1,1} conventions:
      numer/denom = SALL + K1*(OUT2 + SAO),  K1 = (e-1)/2
  - PE transpose + reciprocal + per-partition scale for the final divide.

Transposed fp16 layouts (d-on-partitions) are produced by DMA-transpose of
[rows, 128] fp16 DRAM scratch ([hi|lo] packed), pipelined in 8 column
groups so the PE starts after ~1/8 of the preamble.
"""

import sys

sys.path.insert(0, "/opt/trn_rl_repo")

import numpy as np

import concourse.bass as bass
import concourse.tile as tile
from concourse import bacc, masks, mybir
from concourse.bass_utils import run_bass_kernel_spmd

F32 = mybir.dt.float32
F16 = mybir.dt.float16
AF = mybir.ActivationFunctionType
OP = mybir.AluOpType

N = 8192
D = 64
NCORES = 8
ROWS = N // NCORES          # 1024 rows per core
JT = N // 128               # 64 j-tiles
IT = ROWS // 128            # 8 i-tiles
NG = 16                     # preamble column groups
TPG = JT // NG              # j-tiles per group
R2 = 11.0 * 11.0
K1 = (np.e - 1.0) / 2.0

# experiment flags (consulted at module-build time)
FLAGS = {
    "xw": True,        # build XW
    "hilo": True,      # hi/lo staging + hilod write
    "trans": True,     # W1/W2 transposes
    "sq": True,        # square/reduce/bias
    "iside": True,     # i-side preamble
    "compare": True,   # real compare (False -> memset masks once)
    "sall": True,      # SALL matmuls in loop
    "passc": True,     # pass C matmuls
    "gmm": True,       # G matmuls
    "lag": 2,
}


def _calib_body(nc, tc, pools):
    const, scratch, gpool, mpool, apool, spool = pools
    Gh = gpool.tile([128, 512], mybir.dt.float32, tag="G")
    onec = const.tile([128, 1], F16, tag="calib_one")
    nc.vector.memset(onec[:], 1.0)
    nc.tensor.matmul(Gh[0:1, 0:1], onec[:], onec[:], start=True, stop=True)


def _pre(nc, tc, pools, xf, xi, outd, dram):
    const, scratch, gpool, mpool, apool, spool = pools
    ts = bass.ts

    # ---------------- persistent tiles ----------------
    W1g = [const.tile([128, TPG * 128], F16, name=f"W1_{g}", tag=f"W1_{g}")
           for g in range(NG)]
    ONES128 = const.tile([128, 128], F16, tag="ONES128")
    XW = const.tile([128, 65 * JT], F16, tag="XW")
    R1 = const.tile([128, ROWS], F16, tag="R1")     # [hiT_i; hiT_i]
    R2t = const.tile([128, ROWS], F16, tag="R2")    # [loT_i; v1; v2; v3; 0]
    biasA = const.tile([128, JT], F32, tag="biasA")
    thrD = const.tile([128, JT], F32, tag="thrD")
    ONEC = const.tile([128, 1], F16, tag="ONEC")
    IDN = const.tile([65, 65], F32, tag="IDN")

    # ---------------- preamble: i side (small) ----------------
    r1d = dram.tile([ROWS, 128], F16, tag="r1d")
    r2d = dram.tile([ROWS, 128], F16, tag="r2d")

    if not FLAGS["iside"]:
        nc.vector.memset(R1[:], 0.0)
        nc.vector.memset(R2t[:], 0.0)
        nc.vector.memset(ONEC[:], 1.0)
        nc.vector.memset(ONES128[:], 1.0)
        masks.make_identity(nc, IDN[:])
        _j_side(nc, tc, pools, xf, dram, W1g, XW, biasA, thrD)
        return dict(W1g=W1g, ONES128=ONES128, XW=XW, R1=R1, R2t=R2t,
                    biasA=biasA, thrD=thrD, ONEC=ONEC, IDN=IDN)

    xitp = scratch.tile([128, IT * D], F32, tag="xitp")  # row p*IT + t
    nc.gpsimd.dma_start(xitp[:], xi.rearrange("(p t) d -> p (t d)", p=128))
    xitp3 = xitp[:].rearrange("p (t d) -> p t d", d=D)

    r1sb = scratch.tile([128, IT * 128], F16, tag="r1sb")
    r13 = r1sb[:].rearrange("p (t e) -> p t e", e=128)
    nc.vector.tensor_copy(r13[:, :, 0:D], xitp3)          # hi_i
    r2sb = scratch.tile([128, IT * 128], F16, tag="r2sb")
    r23 = r2sb[:].rearrange("p (t e) -> p t e", e=128)
    nc.gpsimd.memset(r23[:, :, D:128], 0.0)
    nc.vector.tensor_tensor(r23[:, :, 0:D], xitp3, r13[:, :, 0:D], OP.subtract)  # lo_i
    nc.vector.tensor_copy(r13[:, :, D : 2 * D], r13[:, :, 0:D])  # dup hi_i

    s2i = scratch.tile([128, IT * D], F32, tag="s2i")
    nc.scalar.activation(s2i[:], xitp[:], AF.Square)
    sqit = scratch.tile([128, IT], F32, tag="sqit")
    nc.vector.tensor_reduce(sqit[:], s2i[:].rearrange("p (t d) -> p t d", d=D),
                            axis=mybir.AxisListType.X, op=OP.add)
    vfull = scratch.tile([128, IT], F32, tag="vfull")
    nc.vector.tensor_scalar(vfull[:], sqit[:], -0.5, None, OP.mult)
    v1 = scratch.tile([128, IT], F16, tag="v1")
    nc.vector.tensor_copy(v1[:], vfull[:])
    rv1 = scratch.tile([128, IT], F32, tag="rv1")
    nc.vector.tensor_tensor(rv1[:], vfull[:], v1[:], OP.subtract)
    v2 = scratch.tile([128, IT], F16, tag="v2")
    nc.vector.tensor_copy(v2[:], rv1[:])
    rv2 = scratch.tile([128, IT], F32, tag="rv2")
    nc.vector.tensor_tensor(rv2[:], rv1[:], v2[:], OP.subtract)
    v3 = scratch.tile([128, IT], F16, tag="v3")
    nc.vector.tensor_copy(v3[:], rv2[:])
    for k, vk in enumerate((v1, v2, v3)):
        nc.vector.tensor_copy(
            r23[:, :, D + k : D + k + 1],
            vk[:].rearrange("p (t u) -> p t u", u=1),
        )
    nc.gpsimd.dma_start(r1d[:].rearrange("(p t) e -> p (t e)", p=128), r1sb[:])
    nc.gpsimd.dma_start(r2d[:].rearrange("(p t) e -> p (t e)", p=128), r2sb[:])
    nc.sync.dma_start(R1[:], r1d[:], transpose=True)
    nc.sync.dma_start(R2t[:], r2d[:], transpose=True)

    nc.vector.memset(ONEC[:], 1.0)
    nc.vector.memset(ONES128[:], 1.0)
    masks.make_identity(nc, IDN[:])

    # trigger the Sign act-table load early so it overlaps the preamble
    dumm = spool.tile([128, 1], F32, tag="dumm")
    nc.scalar.activation(dumm[:], xitp[:, 0:1], AF.Sign)

    _j_side(nc, tc, pools, xf, dram, W1g, XW, biasA, thrD)

    return dict(W1g=W1g, ONES128=ONES128, XW=XW, R1=R1, R2t=R2t, biasA=biasA,
                thrD=thrD, ONEC=ONEC, IDN=IDN)


def _j_side(nc, tc, pools, xf, dram, W1g, XW, biasA, thrD):
    const, scratch, gpool, mpool, apool, spool = pools
    ts = bass.ts
    hilod = dram.tile([N, 128], F16, tag="hilod")
    # -------- preamble: j side. Phase 1 (copies, ACT ring) --------
    for g in range(NG):
        rows = TPG * 128  # rows covered by this group
        xtp = scratch.tile([128, TPG * D], F32, tag="xtp")  # bufs>1 pool
        # j-tile t holds rows {c*64+t : c in 0..127}; this makes the x load
        # contiguous per partition (the j dimension is only ever summed over,
        # so the relabeling is invisible outside)
        nc.gpsimd.dma_start(
            xtp[:].rearrange("p (t d) -> p t d", d=D),
            xf.rearrange("(p t) d -> p t d", p=128)[:, g * TPG : (g + 1) * TPG, :],
        )
        xtp3 = xtp[:].rearrange("p (t d) -> p t d", d=D)

        # XW slice for this group
        if FLAGS["xw"]:
            xw3 = XW[:].rearrange("p (t e) -> p t e", e=65)[:, g * TPG : (g + 1) * TPG, :]
            nc.vector.tensor_copy(xw3[:, :, 0:D], xtp3)
            nc.gpsimd.memset(xw3[:, :, D : D + 1], 1.0)

        # hi/lo staging -> hilod_g -> two transposes
        if FLAGS["hilo"]:
            hilo = scratch.tile([128, TPG * 128], F16, tag="hilo")
            hl3 = hilo[:].rearrange("p (t e) -> p t e", e=128)
            nc.vector.tensor_copy(hl3[:, :, 0:D], xtp3)
            nc.vector.tensor_tensor(hl3[:, :, D : 2 * D], xtp3, hl3[:, :, 0:D],
                                    OP.subtract)
            hseg = hilod[g * rows : (g + 1) * rows, :]
            nc.gpsimd.dma_start(hseg.rearrange("(t p) e -> p t e", p=128), hl3)
            if FLAGS["trans"]:
                eng = nc.sync if g % 2 == 0 else nc.scalar
                eng.dma_start(W1g[g][:], hseg, transpose=True)

        # sq_j -> bias/thr columns for this group's tiles
        if FLAGS["sq"]:
            s2 = scratch.tile([128, TPG * D], F32, tag="s2")
            nc.scalar.activation(s2[:], xtp[:], AF.Square)
            sl = slice(g * TPG, (g + 1) * TPG)
            nc.vector.tensor_reduce(biasA[:, sl],
                                    s2[:].rearrange("p (t d) -> p t d", d=D),
                                    axis=mybir.AxisListType.X, op=OP.add)
            nc.vector.tensor_scalar(thrD[:, sl], biasA[:, sl], 0.5, -R2 / 2.0,
                                    OP.mult, OP.add)
            nc.vector.tensor_scalar(biasA[:, sl], biasA[:, sl], -0.5, R2 / 2.0,
                                    OP.mult, OP.add)



def _main(nc, tc, pools, outd, env):
    const, scratch, gpool, mpool, apool, spool = pools
    ts = bass.ts
    W1g = env['W1g']; ONES128 = env['ONES128']; XW = env['XW']; R1 = env['R1']
    R2t = env['R2t']; biasA = env['biasA']; thrD = env['thrD']
    ONEC = env['ONEC']; IDN = env['IDN']

    # ---------------- psum accumulators ----------------
    # column block 0:512 always gets the DVE {0,2} mask convention and
    # block 512:1024 the ACT {-1,1} one, so the sign-correction term is
    # just SALL itself:
    #   P[:, 0:512]    = K1*OUT2 + SALL
    #   P[:, 512:1024] = K1*OUT2 + (1+K1)*SALL
    OUT2 = apool.tile([65, ROWS], F32, tag="OUT2")
    SALL = apool.tile([65, 1], F32, tag="SALL")

    # ------- main loop over half j-tiles, pass C lagged by LAG halves ----
    LAG = FLAGS["lag"]
    NH = 2 * JT
    mks = {}
    fixed_mk = None
    if not FLAGS["compare"]:
        fixed_mk = const.tile([128, 512], F16, tag="fixed_mk")
        nc.vector.memset(fixed_mk[:], 1.0)
    for idx in range(NH + LAG):
        if idx < NH:
            t, h = divmod(idx, 2)
            g, tt = divmod(t, TPG)
            cs = slice(512 * h, 512 * (h + 1))
            if FLAGS["gmm"]:
                Gh = gpool.tile([128, 512], F32, tag="G")
                nc.tensor.matmul(Gh[:], W1g[g][:, ts(tt, 128)], R1[:, cs],
                                 start=True, stop=False)
                nc.tensor.matmul(Gh[:], W1g[g][0:64, ts(tt, 128)],
                                 R2t[0:64, cs], start=False, stop=False)
                # v-aug rides PE rows 64-66 concurrently with the pass above
                nc.tensor.matmul(Gh[:], ONES128[64:67, :], R2t[64:67, cs],
                                 start=False, stop=True)
            if FLAGS["compare"]:
                mk = mpool.tile([128, 512], F16, tag="mk")
                if idx % 2 == 0:
                    nc.vector.tensor_scalar(mk[:], Gh[:], thrD[:, t : t + 1],
                                            2.0, OP.is_ge, OP.mult)
                else:
                    nc.scalar.activation(mk[:], Gh[:], AF.Sign,
                                         bias=biasA[:, t : t + 1])
                mks[idx] = mk
            else:
                mks[idx] = fixed_mk
        if idx >= LAG and FLAGS["passc"]:
            jdx = idx - LAG
            t, h = divmod(jdx, 2)
            cs = slice(512 * h, 512 * (h + 1))
            xws = XW[:, 65 * t : 65 * (t + 1)]
            nc.tensor.matmul(OUT2[:, cs], xws, mks.pop(jdx)[:],
                             start=(t == 0), stop=(t == JT - 1))
            if h == 1 and FLAGS["sall"]:
                nc.tensor.matmul(SALL[:], xws, ONEC[:],
                                 start=(t == 0), stop=(t == JT - 1))

    # ---------------- tail (per i-chunk, DVE/ACT alternating) -----------
    sallsb = spool.tile([65, 1], F32, tag="sallsb")
    nc.vector.tensor_copy(sallsb[:], SALL[:])
    b1sb = spool.tile([65, 1], F32, tag="b1sb")
    nc.vector.tensor_scalar(b1sb[:], sallsb[:], 1.0 + K1, None, OP.mult)

    for c in range(IT):
        bap = sallsb if c < IT // 2 else b1sb
        pc = spool.tile([65, 128], F32, tag="pc")
        if c % 2 == 0:
            nc.vector.tensor_scalar(pc[:], OUT2[:, ts(c, 128)], K1, bap[:],
                                    OP.mult, OP.add)
        else:
            nc.scalar.activation(pc[:], OUT2[:, ts(c, 128)], AF.Identity,
                                 bias=bap[:], scale=K1)
        pt = gpool.tile([128, 65], F32, tag="G")
        nc.tensor.transpose(pt[:], pc[:], IDN[:])
        dinv = spool.tile([128, 1], F32, tag="dinv")
        nc.vector.reciprocal(dinv[:], pt[:, D : D + 1])
        ot = spool.tile([128, D], F32, tag="ot")
        nc.vector.tensor_scalar(ot[:], pt[:, 0:D], dinv[:], None, OP.mult)
        nc.sync.dma_start(outd[ts(c, 128), :], ot[:])


def build_module(loop_n=1, scope='full'):
    nc = bacc.Bacc("TRN2", target_bir_lowering=False, debug=False,
                   num_devices=NCORES)
    xf_d = nc.dram_tensor("xf", [N, D], F32, kind="ExternalInput")
    xi_d = nc.dram_tensor("xi", [ROWS, D], F32, kind="ExternalInput")
    out_d = nc.dram_tensor("out", [ROWS, D], F32, kind="ExternalOutput")

    with tile.TileContext(nc) as tc:
        with (
            tc.tile_pool(name="const", bufs=1) as const,
            tc.tile_pool(name="scratch", bufs=2) as scratch,
            tc.tile_pool(name="gpool", bufs=5, space="PSUM") as gpool,
            tc.tile_pool(name="acc", bufs=1, space="PSUM") as apool,
            tc.tile_pool(name="mk", bufs=8) as mpool,
            tc.tile_pool(name="small", bufs=3) as spool,
            tc.tile_pool(name="dram", bufs=3, space="DRAM") as dram,
        ):
            pools = (const, scratch, gpool, mpool, apool, spool)
            args = (nc, tc, pools, xf_d.ap(), xi_d.ap(), out_d.ap(), dram)
            if scope == 'calib':
                with tc.For_i(0, loop_n) as _:
                    _calib_body(nc, tc, pools)
            elif scope == 'pre':
                with tc.For_i(0, loop_n) as _:
                    _pre(*args)
            elif scope == 'main':
                env = _pre(*args)
                with tc.For_i(0, loop_n) as _:
                    _main(nc, tc, pools, out_d.ap(), env)
            elif loop_n == 1:
                env = _pre(*args)
                _main(nc, tc, pools, out_d.ap(), env)
            else:
                with tc.For_i(0, loop_n) as _:
                    env = _pre(*args)
                    _main(nc, tc, pools, out_d.ap(), env)
    nc.finalize()
    return nc


_module_cache = {}


def _get_module(loop_n=1):
    if loop_n not in _module_cache:
        _module_cache[loop_n] = build_module(loop_n)
    return _module_cache[loop_n]


def kernel(x, adj=None):
    x = np.ascontiguousarray(np.asarray(x, dtype=np.float32))
    assert x.shape == (N, D)
    nc = _get_module(1)
    in_maps = [
        {"xf": x, "xi": x[c * ROWS : (c + 1) * ROWS]} for c in range(NCORES)
    ]
    res = run_bass_kernel_spmd(nc, in_maps, core_ids=list(range(NCORES)))
    return np.concatenate([res.results[c]["out"] for c in range(NCORES)], axis=0)



# revision 19
# speedup vs baseline: 1.0880x; 1.0880x over previous
"""BallQueryAttention TRN2 kernel (v2).

Math: reference computes softmax over a binary ball mask (d2 <= R^2), then
mask-softmax @ x.  exp of a 0/1 mask takes only values {1, e}, so

  out[i] = (S + (e-1) * sum_{j in ball(i)} x_j) / (N + (e-1) * cnt_i)

with S = colsum(x).  Sharding: rows (i) across 8 cores, x replicated.

v2 layout strategy (vs v1): no DRAM scratch, no DMA transposes.
  - x loads once as XTP [128, 64*64] f32 (16KB/partition contiguous runs).
  - All transposed operands (W1 [hi_j;lo_j], W2 [hi_j;ones3], R1, R2t) are
    produced by PE transposes of fp16 staging tiles (128x128, ~53ns each)
    with PSUM->SBUF copies on ACT, W2 via Pool copies from W1.
  - Gram: 2 fp16 matmuls per 512-col half:
      Gh  = W1[:,t]^T @ R1[:,cs]      (hh + lh, K=128)
          + W2[0:67,t]^T @ R2t[0:67]  (hl + sq_i aug rows, K=67)
    sq_j rides the fp32 per-partition bias/threshold of the compare.
  - mask compare split Vector (is_ge -> {0,2}) / Scalar (Sign -> {-1,1}).
  - pass C: accumulating [x|1]^T @ mask -> OUT2 [65, 1024] PSUM + SALL.
  - i labeling is transpose-natural (col t*128+p <-> row p*8+t); undone by
    the strided output DMA access pattern.
"""

import sys

sys.path.insert(0, "/opt/trn_rl_repo")

import numpy as np

import concourse.bass as bass
import concourse.tile as tile
from concourse import bacc, masks, mybir
from concourse.bass_utils import run_bass_kernel_spmd

F32 = mybir.dt.float32
F16 = mybir.dt.float16
AF = mybir.ActivationFunctionType
OP = mybir.AluOpType

N = 8192
D = 64
NCORES = 8
ROWS = N // NCORES          # 1024 rows per core
JT = N // 128               # 64 j-tiles
IT = ROWS // 128            # 8 i-tiles
NG = 16                     # j-side groups (4 j-tiles each)
TPG = JT // NG
R2 = 11.0 * 11.0
K1 = (np.e - 1.0) / 2.0
LAG = 2                     # pass-C lag in half-tiles


def _body(nc, tc, pools, xf, xi, outd):
    const, scratch, gpool, mpool, apool, spool, tpool, ppool = pools
    ts = bass.ts

    # ---------------- persistent tiles ----------------
    XTP = const.tile([128, JT * D], F32, tag="XTP")      # x, row p*64+t
    W1 = const.tile([128, N], F16, tag="W1")             # [hiT_j; loT_j]
    W2 = const.tile([67, N], F16, tag="W2")              # [hiT_j; ones3]
    XW = const.tile([128, 65 * JT], F16, tag="XW")       # [x|1] per j-tile
    R1 = const.tile([128, ROWS], F16, tag="R1")          # [hiT_i; hiT_i]
    R2t = const.tile([128, ROWS], F16, tag="R2")         # [loT_i; v1; v2; v3; 0]
    biasA = const.tile([128, JT], F32, tag="biasA")
    thrD = const.tile([128, JT], F32, tag="thrD")
    ONEC = const.tile([128, 1], F16, tag="ONEC")
    IDN = const.tile([65, 65], F32, tag="IDN")
    IDN128 = const.tile([128, 128], F16, tag="IDN128")

    XTP3 = XTP[:].rearrange("p (t d) -> p t d", d=D)
    xf3 = xf.rearrange("(p t) d -> p t d", p=128)

    # ---------------- input DMA (contiguous big runs) ----------------
    # chunk 0 issues first so group-0 staging starts earliest
    xitp = scratch.tile([128, IT * D], F32, tag="xitp")  # row p*8+t
    chunk_tiles = (4, 12, 16, 16, 16)    # j-tiles per chunk (sums to 64)
    pos = 0
    for c4, w in enumerate(chunk_tiles):
        eng = nc.sync if c4 % 2 == 0 else nc.scalar
        sl = slice(pos, pos + w)
        eng.dma_start(XTP3[:, sl, :], xf3[:, sl, :])
        if c4 == 0:
            nc.scalar.dma_start(xitp[:],
                                xi.rearrange("(p t) d -> p (t d)", p=128))
        pos += w

    # ---------------- small consts ----------------
    nc.vector.memset(ONEC[:], 1.0)
    masks.make_identity(nc, IDN[:])
    masks.make_identity(nc, IDN128[:])
    # (W2/XW ones rows are memset per-group inside emit_group to keep the
    # Pool queue free early)

    # trigger the act-table load early so Sign is ready for the main loop
    dumm = spool.tile([128, 1], F32, tag="dumm")
    nc.scalar.activation(dumm[:], ONEC[:], AF.Sign)

    def emit_iside():
        # i-side staging + PE transposes.  v-chain on Pool keeps DVE free
        # for j-side staging.
        xitp3 = xitp[:].rearrange("p (t d) -> p t d", d=D)
        r1sb = scratch.tile([128, IT * 128], F16, tag="r1sb")
        r13 = r1sb[:].rearrange("p (t e) -> p t e", e=128)
        nc.vector.tensor_copy(r13[:, :, 0:D], xitp3)                   # hi_i
        nc.vector.tensor_copy(r13[:, :, D : 2 * D], r13[:, :, 0:D])    # dup hi_i
        r2sb = scratch.tile([128, IT * 128], F16, tag="r2sb")
        r23 = r2sb[:].rearrange("p (t e) -> p t e", e=128)
        nc.gpsimd.memset(r23[:, :, D + 3 : 128], 0.0)
        nc.vector.tensor_tensor(r23[:, :, 0:D], xitp3, r13[:, :, 0:D],
                                OP.subtract)                           # lo_i

        s2i = scratch.tile([128, IT * D], F32, tag="s2i")
        nc.scalar.activation(s2i[:], xitp[:], AF.Square)
        sqit = scratch.tile([128, IT], F32, tag="sqit")
        nc.vector.tensor_reduce(sqit[:],
                                s2i[:].rearrange("p (t d) -> p t d", d=D),
                                axis=mybir.AxisListType.X, op=OP.add)
        vfull = scratch.tile([128, IT], F32, tag="vfull")
        nc.gpsimd.tensor_scalar(vfull[:], sqit[:], -0.5, None, OP.mult)
        v1 = scratch.tile([128, IT], F16, tag="v1")
        nc.gpsimd.tensor_copy(v1[:], vfull[:])
        rv1 = scratch.tile([128, IT], F32, tag="rv1")
        nc.gpsimd.tensor_tensor(rv1[:], vfull[:], v1[:], OP.subtract)
        v2 = scratch.tile([128, IT], F16, tag="v2")
        nc.gpsimd.tensor_copy(v2[:], rv1[:])
        rv2 = scratch.tile([128, IT], F32, tag="rv2")
        nc.gpsimd.tensor_tensor(rv2[:], rv1[:], v2[:], OP.subtract)
        v3 = scratch.tile([128, IT], F16, tag="v3")
        nc.gpsimd.tensor_copy(v3[:], rv2[:])
        for k, vk in enumerate((v1, v2, v3)):
            nc.gpsimd.tensor_copy(
                r23[:, :, D + k : D + k + 1],
                vk[:].rearrange("p (t u) -> p t u", u=1),
            )

        # PE transposes: [128p, 128e] tiles -> [128e, 128p] -> R1/R2t
        for half in range(2):
            fs = slice(half * 512, (half + 1) * 512)
            tp1 = tpool.tile([128, 512], F16, tag="tp")
            for q in range(4):
                t8 = half * 4 + q
                nc.tensor.transpose(tp1[:, ts(q, 128)], r1sb[:, ts(t8, 128)],
                                    IDN128[:])
            nc.scalar.activation(R1[:, fs], tp1[:], AF.Identity)
            tp2 = tpool.tile([128, 512], F16, tag="tp")
            for q in range(4):
                t8 = half * 4 + q
                nc.tensor.transpose(tp2[:, ts(q, 128)], r2sb[:, ts(t8, 128)],
                                    IDN128[:])
            nc.scalar.activation(R2t[:, fs], tp2[:], AF.Identity)

    # ---------------- psum accumulators ----------------
    # column block 0:512 gets the DVE {0,2} mask convention and 512:1024 the
    # ACT {-1,1} one:
    #   P[:, 0:512]    = K1*OUT2 + SALL
    #   P[:, 512:1024] = K1*OUT2 + (1+K1)*SALL
    OUT2 = apool.tile([65, ROWS], F32, tag="OUT2")
    # tail transpose scratch shares a PSUM tile with the SALL accumulator
    PT8 = ppool.tile([128, IT * 65 + 4], F32, tag="PT8")
    SALL = PT8[0:65, IT * 65 : IT * 65 + 1]

    # ---------------- j-side groups interleaved with main halves --------
    NH = 2 * JT
    mks = {}
    emitted = 0

    def emit_main(upto):
        # emit main-loop half-tiles with pass C lagged by LAG halves
        nonlocal emitted
        while emitted < upto:
            idx = emitted
            if idx < NH:
                t, h = divmod(idx, 2)
                cs = slice(512 * h, 512 * (h + 1))
                Gh = gpool.tile([128, 512], F32, tag="G")
                nc.tensor.matmul(Gh[:], W1[:, ts(t, 128)], R1[:, cs],
                                 start=True, stop=False)
                nc.tensor.matmul(Gh[:], W2[0:67, ts(t, 128)], R2t[0:67, cs],
                                 start=False, stop=True)
                mk = mpool.tile([128, 512], F16, tag="mk")
                if idx % 2 == 0:
                    nc.vector.tensor_scalar(mk[:], Gh[:], thrD[:, t : t + 1],
                                            2.0, OP.is_ge, OP.mult)
                else:
                    nc.scalar.activation(mk[:], Gh[:], AF.Sign,
                                         bias=biasA[:, t : t + 1])
                mks[idx] = mk
            if idx >= LAG:
                jdx = idx - LAG
                t, h = divmod(jdx, 2)
                cs = slice(512 * h, 512 * (h + 1))
                xws = XW[:, 65 * t : 65 * (t + 1)]
                nc.tensor.matmul(OUT2[:, cs], xws, mks.pop(jdx)[:],
                                 start=(t == 0), stop=(t == JT - 1))
                if h == 1:
                    nc.tensor.matmul(SALL[:], xws, ONEC[:],
                                     start=(t == 0), stop=(t == JT - 1))
            emitted += 1

    def emit_group(g):
        gt = slice(g * TPG, (g + 1) * TPG)          # j-tiles of this group
        src = XTP3[:, gt, :]                        # [128, 4, 64] f32

        # [x|1] tile for pass C (Pool, SBUF->SBUF f32->f16)
        xw3 = XW[:].rearrange("p (t e) -> p t e", e=65)[:, gt, :]
        nc.gpsimd.tensor_copy(xw3[:, :, 0:D], src)
        nc.gpsimd.memset(xw3[:, :, D : D + 1], 1.0)

        # hi/lo staging in natural layout
        hilo = scratch.tile([128, TPG * 128], F16, tag="hilo")
        hl3 = hilo[:].rearrange("p (t e) -> p t e", e=128)
        nc.vector.tensor_copy(hl3[:, :, 0:D], src)
        nc.vector.tensor_tensor(hl3[:, :, D : 2 * D], src, hl3[:, :, 0:D],
                                OP.subtract)

        # sq_j -> bias/thr columns (Square on ACT, strided reduce on DVE)
        s2 = scratch.tile([128, TPG * D], F32, tag="s2")
        nc.scalar.activation(s2[:].rearrange("p (t d) -> p t d", d=D), src,
                             AF.Square)
        nc.vector.tensor_reduce(biasA[:, gt],
                                s2[:].rearrange("p (t d) -> p t d", d=D),
                                axis=mybir.AxisListType.X, op=OP.add)
        nc.vector.tensor_scalar(thrD[:, gt], biasA[:, gt], 0.5, -R2 / 2.0,
                                OP.mult, OP.add)
        nc.vector.tensor_scalar(biasA[:, gt], biasA[:, gt], -0.5, R2 / 2.0,
                                OP.mult, OP.add)

        # PE transposes -> W1 (ACT copy), W2 hi rows (Pool copy from W1)
        wsl = slice(g * TPG * 128, (g + 1) * TPG * 128)
        tpw = tpool.tile([128, 512], F16, tag="tp")
        for q in range(TPG):
            nc.tensor.transpose(tpw[:, ts(q, 128)], hilo[:, ts(q, 128)], IDN128[:])
        nc.scalar.activation(W1[:, wsl], tpw[:], AF.Identity)
        nc.gpsimd.tensor_copy(W2[0:64, wsl], W1[0:64, wsl])
        nc.gpsimd.memset(W2[64:67, wsl], 1.0)

    emit_group(0)            # group 0 first: its chunk lands earliest
    emit_iside()
    for g in range(1, NG):
        emit_group(g)
        # overlap: emit main halves for groups that are fully ready
        emit_main(g * TPG * 2)
    emit_main(NH + LAG)

    # ---------------- tail -----------------------------------------
    # all pc scales first (independent, DVE/ACT split), then PE transposes
    # into one wide PSUM tile, one strided reciprocal, per-chunk final
    # scales, and a single contiguous-run output DMA (2KB/partition).
    sallsb = spool.tile([65, 1], F32, tag="sallsb")
    nc.vector.tensor_copy(sallsb[:], SALL[:])
    b1sb = spool.tile([65, 1], F32, tag="b1sb")
    nc.vector.tensor_scalar(b1sb[:], sallsb[:], 1.0 + K1, None, OP.mult)

    pcs = []
    for c in range(IT):
        bap = sallsb if c < IT // 2 else b1sb
        pc = spool.tile([65, 128], F32, tag=f"pc{c % 4}")
        if c % 2 == 0:
            nc.vector.tensor_scalar(pc[:], OUT2[:, ts(c, 128)], K1, bap[:],
                                    OP.mult, OP.add)
        else:
            nc.scalar.activation(pc[:], OUT2[:, ts(c, 128)], AF.Identity,
                                 bias=bap[:], scale=K1)
        pcs.append(pc)
    for c in range(IT):
        nc.tensor.transpose(PT8[:, ts(c, 65)], pcs[c][:], IDN[:])
    pt3 = PT8[:, 0 : IT * 65].rearrange("p (t e) -> p t e", e=65)
    dinv = spool.tile([128, IT], F32, tag="dinv")
    nc.vector.reciprocal(dinv[:], pt3[:, :, D])
    ot = spool.tile([128, IT * D], F32, tag="ot")
    for c in range(IT):
        if c % 2 == 0:
            nc.vector.tensor_scalar(ot[:, ts(c, D)], pt3[:, c, 0:D],
                                    dinv[:, c : c + 1], None, OP.mult)
        else:
            nc.scalar.activation(ot[:, ts(c, D)], pt3[:, c, 0:D], AF.Identity,
                                 scale=dinv[:, c : c + 1])
    nc.sync.dma_start(outd.rearrange("(p t) d -> p (t d)", p=128), ot[:])


def build_module(loop_n=1, scope="full"):
    nc = bacc.Bacc("TRN2", target_bir_lowering=False, debug=False,
                   num_devices=NCORES)
    xf_d = nc.dram_tensor("xf", [N, D], F32, kind="ExternalInput")
    xi_d = nc.dram_tensor("xi", [ROWS, D], F32, kind="ExternalInput")
    out_d = nc.dram_tensor("out", [ROWS, D], F32, kind="ExternalOutput")

    with tile.TileContext(nc) as tc:
        with (
            tc.tile_pool(name="const", bufs=1) as const,
            tc.tile_pool(name="scratch", bufs=2) as scratch,
            tc.tile_pool(name="gpool", bufs=3, space="PSUM") as gpool,
            tc.tile_pool(name="mk", bufs=8) as mpool,
            tc.tile_pool(name="acc", bufs=1, space="PSUM") as apool,
            tc.tile_pool(name="small", bufs=3) as spool,
            tc.tile_pool(name="tpose", bufs=1, space="PSUM") as tpool,
            tc.tile_pool(name="ptp", bufs=1, space="PSUM") as ppool,
        ):
            pools = (const, scratch, gpool, mpool, apool, spool, tpool, ppool)
            args = (nc, tc, pools, xf_d.ap(), xi_d.ap(), out_d.ap())
            if loop_n == 1:
                _body(*args)
            else:
                with tc.For_i(0, loop_n) as _:
                    _body(*args)
    nc.finalize()
    return nc


_module_cache = {}


def _get_module(loop_n=1):
    if loop_n not in _module_cache:
        _module_cache[loop_n] = build_module(loop_n)
    return _module_cache[loop_n]


def kernel(x, adj=None):
    x = np.ascontiguousarray(np.asarray(x, dtype=np.float32))
    assert x.shape == (N, D)
    nc = _get_module(1)
    in_maps = [
        {"xf": x, "xi": x[c * ROWS : (c + 1) * ROWS]} for c in range(NCORES)
    ]
    res = run_bass_kernel_spmd(nc, in_maps, core_ids=list(range(NCORES)))
    return np.concatenate([res.results[c]["out"] for c in range(NCORES)], axis=0)
